# revision 15
# baseline (speedup 1.0000x reference)
"""GCN (2 dense + 3 sparse layers + log_softmax) on 8 Trainium2 NeuronCores.

Strategy: each graph aggregation A_norm @ H runs densely on the PE as
out_T[f, t] = sum_s H'[s, f] * B^T[s, t] with B the count-valued adjacency in
fp8 (exact small integers) and H' the diag-scaled features in fp16
(stationary operand).  Nodes (dst) are row-sharded 8 ways; each layer's
feature block is exchanged via AllGather (half-exchanges for the 32-wide
layers where Mesh latency is low; one exchange for the wider layers).

The two adjacency matrices are RESIDENT in SBUF: bden streams from HBM once
(used by l1 + l2), then bsp streams once into the same region as l2
consumes it (used by l3/l4/l5).  All h_full fill DMAs ride the same sync
HWDGE queue as the B streams, interleaved so small fill packets are never
starved by stream packets on the shared SDMA engines.
"""

import os
import numpy as np
import ml_dtypes

import concourse.bacc as bacc
import concourse.mybir as mybir
import concourse.tile as tile
from concourse.bass_utils import run_bass_kernel_spmd

# ---- problem constants ----
N = 12000
NP = 12288         # padded nodes (96 * 128)
NCORES = 8
NLOC = NP // NCORES            # 1536 rows per core
KC = NP // 128                 # 96 k-chunks
MC = NLOC // 128               # 12 local row chunks
MH = MC // 2                   # half split (6 chunks)
NT = NLOC // 512               # 3 psum col tiles
NGRP = 8                       # B-stream DMA groups
GC = KC // NGRP                # 12 chunks per group
GW = GC * NLOC                 # group width in cden columns
F_IN = 512
CLS = 6

F8 = mybir.dt.float8e4
F16 = mybir.dt.float16
F32 = mybir.dt.float32
NP_F8 = ml_dtypes.float8_e4m3
NP_F16 = np.float16

D1, D2, D3, D4, D5 = 32, 32, 64, 128, 32   # aggregation widths per layer

# B consumption order: A-half (m<MH of every rank) first, then B-half, so
# each layer's second half fill hides under first-half aggregation.
# B matrices are stored in this order host-side -> sequential DMA stream.
CHUNK_ORDER = ([c * MC + m for c in range(NCORES) for m in range(MH)]
               + [c * MC + m for c in range(NCORES) for m in range(MH, MC)])

_cached = {}


def _build_program():
    nc = bacc.Bacc("TRN2", target_bir_lowering=False, debug=False,
                   num_devices=NCORES)

    bden = nc.dram_tensor("bden", [NGRP, 128, GW], F8, kind="ExternalInput")
    bsp = nc.dram_tensor("bsp", [NGRP, 128, GW], F8, kind="ExternalInput")
    featT = nc.dram_tensor("featT", [4, 128, NLOC], F16, kind="ExternalInput")
    w1 = nc.dram_tensor("w1", [4, 128, 32], F16, kind="ExternalInput")
    w12b = nc.dram_tensor("w12b", [33, 64], F16, kind="ExternalInput")
    w13b = nc.dram_tensor("w13b", [65, 128], F16, kind="ExternalInput")
    w14 = nc.dram_tensor("w14", [128, 128], F16, kind="ExternalInput")
    w2 = nc.dram_tensor("w2", [128, CLS], F16, kind="ExternalInput")
    biases_pp = nc.dram_tensor("biases_pp", [128, 3], F32, kind="ExternalInput")
    dis_repl = nc.dram_tensor("dis_repl", [128, NLOC], F32, kind="ExternalInput")
    dinv_repl = nc.dram_tensor("dinv_repl", [128, NLOC], F32, kind="ExternalInput")
    dis_pp = nc.dram_tensor("dis_pp", [128, MC], F32, kind="ExternalInput")
    dinv_pp = nc.dram_tensor("dinv_pp", [128, MC], F32, kind="ExternalInput")
    ident16 = nc.dram_tensor("ident16", [128, 128], F16, kind="ExternalInput")
    ident32 = nc.dram_tensor("ident32", [128, 128], F32, kind="ExternalInput")
    out = nc.dram_tensor("out", [NLOC, CLS], F32, kind="ExternalOutput")

    AG = mybir.AluOpType
    AF = mybir.ActivationFunctionType
    RG = [list(range(NCORES))]

    with tile.TileContext(nc) as tc:
        with (
            tc.tile_pool(name="const", bufs=1) as cpool,
            tc.tile_pool(name="dscale", bufs=1) as dpool_s,
            tc.tile_pool(name="resident", bufs=1) as hpool,
            tc.tile_pool(name="fq", bufs=2) as fpool,
            tc.tile_pool(name="work", bufs=1) as wpool,
            tc.tile_pool(name="post", bufs=3) as ppool,
            tc.tile_pool(name="small", bufs=2) as spool,
            tc.tile_pool(name="agg", bufs=4, space="PSUM") as aggp,
            tc.tile_pool(name="wmm", bufs=2, space="PSUM") as wmmp,
            tc.tile_pool(name="tp", bufs=1, space="PSUM") as tpp,
            tc.tile_pool(name="dram", bufs=1, space="DRAM") as dpool,
        ):
            # ---------- early constants (l1 path only) ----------
            w1_sb = cpool.tile([128, 4 * 32], F16, tag="w1")
            nc.scalar.dma_start(w1_sb[:].rearrange("p (c j) -> p c j", c=4),
                                w1.ap().rearrange("c p j -> p c j"))
            bias_sb = cpool.tile([128, 3], F32, tag="bias")
            nc.scalar.dma_start(bias_sb[:], biases_pp[:, :])
            dispp_sb = cpool.tile([128, MC], F32, tag="dispp")
            nc.scalar.dma_start(dispp_sb[:], dis_pp[:, :])
            id16_sb = cpool.tile([128, 128], F16, tag="id16")
            nc.scalar.dma_start(id16_sb[:], ident16[:, :])

            # resident adjacency (bden first, overwritten by bsp during l2)
            cden = hpool.tile([128, KC * NLOC], F8, tag="cden")
            # h_full holds up to 64 feature cols per chunk; layer 4 (d=128)
            # runs as two 64-wide f-half passes re-filled from DRAM.
            h_full = hpool.tile([128, KC * 64], F16, tag="hfull")

            def half_exchange(hloc, d, lname, half, whole=False):
                """AllGather MH (or MC if whole) chunks of the local block;
                returns the gathered DRAM buffer."""
                nch = MC if whole else MH
                o = half * MH * d
                w = nch * d
                bin_t = dpool.tile([128, w], F16, tag=f"agi{lname}{half}")
                bout_t = dpool.tile([NCORES, 128, w], F16,
                                    tag=f"ago{lname}{half}", addr_space="Shared")
                nc.scalar.dma_start(bin_t[:], hloc[:, o:o + w])
                nc.gpsimd.collective_compute(
                    "AllGather", AG.bypass, replica_groups=RG,
                    ins=[bin_t.opt()], outs=[bout_t.opt()],
                )
                return bout_t

            def fill_hfull(bout_t, d_src, half, f0, fw, src_half=None):
                """Scatter gathered blocks into h_full chunk positions
                c*MC+m (fw cols per chunk), source cols f0:f0+fw of a
                d_src-wide gather.  src_half: m-offset inside bout when it
                holds a whole-MC gather.  Rides the sync HWDGE queue."""
                view = h_full[:, 0:KC * fw].rearrange("p (c x) -> p c x",
                                                      c=NCORES)
                dst = view[:, :, (half * MH) * fw:(half * MH + MH) * fw]
                mo = 0 if src_half is None else src_half * MH * d_src
                if f0 == 0 and fw == d_src:
                    nc.sync.dma_start(
                        dst, bout_t[:, :, mo:mo + MH * fw].rearrange(
                            "c p w -> p c w"))
                else:
                    # f-sliced case needs 4D APs, unsupported by DMA ->
                    # one 3D transfer per source core.
                    for c in range(NCORES):
                        k0 = c * MC + half * MH
                        nc.sync.dma_start(
                            h_full[:, k0 * fw:(k0 + MH) * fw].rearrange(
                                "p (m f) -> p m f", f=fw),
                            bout_t[c, :, mo:mo + MH * d_src].rearrange(
                                "p (m f) -> p m f",
                                f=d_src)[:, :, f0:f0 + fw])

            def bpass(d, lname):
                """Aggregation pass over the resident adjacency, consuming
                stream positions in CHUNK_ORDER (A-half chunks first)."""
                P4 = 128 // d    # col-group packing factor
                aggs = [aggp.tile([128, 512], F32, tag="agg",
                                  name=f"agg_{lname}_{i}") for i in range(NT)]
                for i in range(KC):
                    k = CHUNK_ORDER[i]      # global chunk at stream pos i
                    q = i % P4               # PE column group
                    lhs = h_full[:, k * d:(k + 1) * d]
                    for t in range(NT):
                        nc.tensor.matmul(
                            aggs[t][q * d:(q + 1) * d, :], lhs,
                            cden[:, i * NLOC + t * 512:
                                 i * NLOC + (t + 1) * 512],
                            start=(i < P4), stop=(i >= KC - P4),
                            tile_position=(0, q * d),
                        )
                return aggs

            def gsum(aggs, t, d, lname, pbase=0):
                """Sum the P4 col-group partials of psum tile t -> [d, 512]
                placed at SBUF partitions pbase:pbase+d (SB operands of a
                DVE op must share a start partition; PSUM ones are free)."""
                P4 = 128 // d
                a = aggs[t]
                if P4 == 1:
                    return a
                tmp = spool.tile([128, 512], F32, tag="gsum",
                                 name=f"gs_{lname}_{t}")
                view = tmp[pbase:pbase + d, :]
                nc.vector.tensor_copy(view, a[pbase:pbase + d, :])
                for q in range(P4):
                    if q * d == pbase:
                        continue
                    nc.vector.tensor_tensor(view, view,
                                            a[q * d:(q + 1) * d, :], op=AG.add)
                return tmp[pbase:pbase + d, :]

            # tiny warm-up collective: pays the first-op overhead and the
            # device-alignment barrier before the real exchanges need it
            warm_i = dpool.tile([128, 8], F16, tag="warm_i")
            warm_o = dpool.tile([NCORES, 128, 8], F16, tag="warm_o",
                                addr_space="Shared")
            nc.gpsimd.collective_compute(
                "AllGather", AG.bypass, replica_groups=RG,
                ins=[warm_i.opt()], outs=[warm_o.opt()])

            # bden stream starts immediately; l1 consumes groups as they land
            for g in range(NGRP):
                nc.sync.dma_start(cden[:, g * GW:(g + 1) * GW], bden[g, :, :])

            # ============ L1 local transform: H'1 = dis * (X0 @ W1) ==========
            # kc-outer so the feature matrix streams in quarters (6 KB SBUF).
            # Each (kc, m) matmul is a self-contained group into its own psum
            # column range; the kc-reduction is summed on the vector engine.
            h1loc = wpool.tile([128, MC * D1], F16, tag="h1loc")
            t1k = []
            for kc in range(4):
                fq = fpool.tile([128, NLOC], F16, tag="fq", name=f"fq{kc}")
                nc.scalar.dma_start(fq[:], featT[kc, :, :])
                tk = aggp.tile([128, MC * 32], F32, tag="agg", name=f"t1k{kc}")
                t1k.append(tk)
                for m in range(MC):
                    nc.tensor.matmul(
                        tk[:, m * 32:(m + 1) * 32],
                        fq[:, m * 128:(m + 1) * 128],
                        w1_sb[:, kc * 32:(kc + 1) * 32],
                        start=True, stop=True,
                    )
            bouts = {}
            for half in range(2):
                cols = slice(half * MH * 32, (half + 1) * MH * 32)
                s01 = spool.tile([128, MH * 32], F32, tag="t1s",
                                 name=f"s01_{half}")
                nc.vector.tensor_copy(s01[:, :], t1k[0][:, cols])
                for kc in range(1, 4):
                    nc.vector.tensor_tensor(s01[:, :], s01[:, :],
                                            t1k[kc][:, cols], op=AG.add)
                for m in range(half * MH, (half + 1) * MH):
                    col = (m % MH) * 32
                    nc.vector.tensor_scalar_mul(
                        h1loc[:, m * D1:(m + 1) * D1],
                        s01[:, col:col + 32],
                        dispp_sb[:, m:m + 1])
                bouts[("l1", half)] = half_exchange(h1loc, D1, "l1", half)
            for half in range(2):
                fill_hfull(bouts[("l1", half)], D1, half, 0, D1)

            # dis/dinv replicated scale rows (loaded in the quiet window
            # before the bsp stream starts)
            disr_sb = dpool_s.tile([128, NLOC], F32, tag="dsc", name="disr")
            nc.scalar.dma_start(disr_sb[:], dis_repl[:, :])
            dinvr_sb = dpool_s.tile([128, NLOC], F32, tag="dinv", name="dinvr")
            nc.scalar.dma_start(dinvr_sb[:], dinv_repl[:, :])

            # ============ L1 agg + post: x1 = relu(dis*G1 + b1) ==============
            aggs = bpass(D1, "l1")
            x1p = ppool.tile([32, NLOC], F16, tag="post", name="x1p")
            h2loc = wpool.tile([128, MC * D2], F16, tag="h2loc")
            tp1 = tpp.tile([128, MC * 32], F16, tag="tp16")
            for half in range(2):
                for t in ((0, 1) if half == 0 else (2,)):
                    sl = slice(t * 512, (t + 1) * 512)
                    g1s = gsum(aggs, t, D1, "l1")
                    nc.vector.tensor_tensor(
                        g1s[:, :], g1s[:, :], disr_sb[0:32, sl], op=AG.mult)
                    x1t = spool.tile([32, 512], F16, tag="x1t", name=f"x1t_{t}")
                    nc.scalar.activation(x1t[:, :], g1s[:, :], AF.Relu,
                                         bias=bias_sb[0:32, 0:1])
                    nc.vector.tensor_tensor(
                        x1p[:, sl], x1t[:, :], disr_sb[0:32, sl], op=AG.mult)
                for m in range(half * MH, (half + 1) * MH):
                    nc.tensor.transpose(
                        tp1[:, m * 32:(m + 1) * 32],
                        x1p[:, m * 128:(m + 1) * 128], id16_sb[0:32, 0:32])
                o = half * MH * D2
                nc.vector.tensor_copy(h2loc[:, o:o + MH * D2],
                                      tp1[:, o:o + MH * D2])
                bouts[("l2", half)] = half_exchange(h2loc, D2, "l2", half)
            for half in range(2):
                fill_hfull(bouts[("l2", half)], D2, half, 0, D2)

            # ============ L2: agg + x2 = relu(dis*G2 @ W12 + b12) ============
            w12_sb = cpool.tile([33, 64], F16, tag="w12")
            nc.scalar.dma_start(w12_sb[:], w12b[:, :])
            dinvpp_sb = cpool.tile([128, MC], F32, tag="dinvpp")
            nc.scalar.dma_start(dinvpp_sb[:], dinv_pp[:, :])
            aggs = bpass(D2, "l2")
            # bsp overwrites each cden group as soon as l2's matmuls have
            # consumed it (WAR deps per slice).  Groups are emitted in
            # slices with the l3 fills interleaved between them, so the
            # fills get full SDMA bandwidth at the moment they unblock.
            for g in range(4):
                nc.sync.dma_start(cden[:, g * GW:(g + 1) * GW], bsp[g, :, :])
            g2p = ppool.tile([33, NLOC], F16, tag="post", name="g2p")
            nc.vector.memset(g2p[32:33, :], 1.0)
            h3loc = wpool.tile([128, MC * D3], F16, tag="h3loc")
            for half in range(2):
                for t in ((0, 1) if half == 0 else (2,)):
                    sl = slice(t * 512, (t + 1) * 512)
                    nc.vector.tensor_tensor(
                        g2p[0:32, sl], gsum(aggs, t, D2, "l2"), disr_sb[0:32, sl],
                        op=AG.mult)
                for m in range(half * MH, (half + 1) * MH):
                    xp = wmmp.tile([128, 64], F32, tag="wmm", name=f"x2_{m}")
                    nc.tensor.matmul(xp[:, :], g2p[:, m * 128:(m + 1) * 128],
                                     w12_sb[:, :], start=True, stop=True)
                    nc.vector.tensor_scalar(
                        h3loc[:, m * D3:(m + 1) * D3], xp[:, :],
                        0.0, dinvpp_sb[:, m:m + 1], op0=AG.max, op1=AG.mult)
            # one whole-block exchange for l3 (two RDH ops would serialize)
            bout3 = half_exchange(h3loc, D3, "l3", 0, whole=True)
            fill_hfull(bout3, D3, 0, 0, D3, src_half=0)
            for g in range(4, 6):
                nc.sync.dma_start(cden[:, g * GW:(g + 1) * GW], bsp[g, :, :])
            fill_hfull(bout3, D3, 1, 0, D3, src_half=1)
            for g in range(6, NGRP):
                nc.sync.dma_start(cden[:, g * GW:(g + 1) * GW], bsp[g, :, :])

            # ============ L3: agg + x3 = relu(dinv*G3 @ W13 + b13) ===========
            w13_sb = cpool.tile([65, 128], F16, tag="w13")
            nc.scalar.dma_start(w13_sb[:], w13b[:, :])
            aggs = bpass(D3, "l3")
            g3p = ppool.tile([65, NLOC], F16, tag="post", name="g3p")
            nc.vector.memset(g3p[64:65, :], 1.0)
            h4loc = wpool.tile([128, MC * D4], F16, tag="h4loc")
            for half in range(2):
                for t in ((0, 1) if half == 0 else (2,)):
                    sl = slice(t * 512, (t + 1) * 512)
                    nc.vector.tensor_tensor(
                        g3p[0:64, sl], gsum(aggs, t, D3, "l3"), dinvr_sb[0:64, sl],
                        op=AG.mult)
                for m in range(half * MH, (half + 1) * MH):
                    xp = wmmp.tile([128, 128], F32, tag="wmm", name=f"x3_{m}")
                    nc.tensor.matmul(xp[:, :], g3p[:, m * 128:(m + 1) * 128],
                                     w13_sb[:, :], start=True, stop=True)
                    nc.vector.tensor_scalar(
                        h4loc[:, m * D4:(m + 1) * D4], xp[:, :],
                        0.0, dinvpp_sb[:, m:m + 1], op0=AG.max, op1=AG.mult)
            # one whole-block exchange for l4 (f-halves filled separately)
            bout4 = half_exchange(h4loc, D4, "l4", 0, whole=True)
            for half in range(2):
                fill_hfull(bout4, D4, half, 0, 64, src_half=half)

            # ===== L4: agg + x4T = relu(dinv*G4 @ W14 + b14)  (transposed) ===
            # ===== L5a: H'5T = dinv * (x4 @ W2), transpose, exchange =========
            # d=128 runs as two 64-wide f-half passes so h_full stays 12 KB.
            w14_sb = cpool.tile([128, 128], F16, tag="w14")
            nc.scalar.dma_start(w14_sb[:], w14[:, :])
            w2_sb = cpool.tile([128, CLS], F16, tag="w2")
            nc.scalar.dma_start(w2_sb[:], w2[:, :])
            aggs = bpass(64, "l4a")
            g4p = ppool.tile([128, NLOC], F16, tag="post", name="g4p")
            for t in range(NT):
                sl = slice(t * 512, (t + 1) * 512)
                nc.vector.tensor_tensor(
                    g4p[0:64, sl], gsum(aggs, t, 64, "l4a"),
                    dinvr_sb[0:64, sl], op=AG.mult)
            for half in range(2):
                fill_hfull(bout4, D4, half, 64, 64, src_half=half)
            aggs = bpass(64, "l4b")
            x4T = ppool.tile([128, NLOC], F16, tag="post", name="x4T")
            h5T = ppool.tile([32, NLOC], F16, tag="post", name="h5T")
            nc.vector.memset(h5T[0:32, :], 0.0)
            h5loc = wpool.tile([128, MC * D5], F16, tag="h5loc")
            tp5 = tpp.tile([128, MC * 32], F16, tag="tp16")
            for half in range(2):
                for t in ((0, 1) if half == 0 else (2,)):
                    sl = slice(t * 512, (t + 1) * 512)
                    nc.vector.tensor_tensor(
                        g4p[64:128, sl], gsum(aggs, t, 64, "l4b", pbase=64),
                        dinvr_sb[64:128, sl], op=AG.mult)
                    x4p = wmmp.tile([128, 512], F32, tag="wmm", name=f"x4_{t}")
                    nc.tensor.matmul(x4p[:, :], w14_sb[:, :], g4p[:, sl],
                                     start=True, stop=True)
                    nc.scalar.activation(x4T[:, sl], x4p[:, :], AF.Relu,
                                         bias=bias_sb[:, 1:2])
                    t5 = wmmp.tile([CLS, 512], F32, tag="wmm", name=f"t5_{t}")
                    nc.tensor.matmul(t5[:, :], w2_sb[:, :], x4T[:, sl],
                                     start=True, stop=True)
                    nc.vector.tensor_tensor(
                        h5T[0:CLS, sl], t5[:, :], dinvr_sb[0:CLS, sl],
                        op=AG.mult)
                for m in range(half * MH, (half + 1) * MH):
                    nc.tensor.transpose(
                        tp5[:, m * 32:(m + 1) * 32],
                        h5T[:, m * 128:(m + 1) * 128], id16_sb[0:32, 0:32])
                o = half * MH * D5
                nc.vector.tensor_copy(h5loc[:, o:o + MH * D5],
                                      tp5[:, o:o + MH * D5])
                bouts[("l5", half)] = half_exchange(h5loc, D5, "l5", half)
            for half in range(2):
                fill_hfull(bouts[("l5", half)], D5, half, 0, D5)

            # ============ L5b: agg + z = dinv*G5 + b2, log_softmax ===========
            id32_sb = cpool.tile([128, 128], F32, tag="id32")
            nc.scalar.dma_start(id32_sb[:], ident32[:, :])
            aggs = bpass(D5, "l5")
            zt = wpool.tile([32, NLOC], F32, tag="zt")
            nc.vector.memset(zt[0:32, :], 0.0)
            for t in range(NT):
                sl = slice(t * 512, (t + 1) * 512)
                nc.vector.tensor_tensor(
                    zt[0:CLS, sl], gsum(aggs, t, D5, "l5")[0:CLS, :],
                    dinvr_sb[0:CLS, sl], op=AG.mult)
                nc.vector.tensor_scalar_add(
                    zt[0:CLS, sl], zt[0:CLS, sl], bias_sb[0:CLS, 2:3])
            ztp = tpp.tile([128, MC * 32], F32, tag="tp32")
            outsb = wpool.tile([128, MC * CLS], F32, tag="outsb")
            for m in range(MC):
                nc.tensor.transpose(
                    ztp[:, m * 32:(m + 1) * 32],
                    zt[:, m * 128:(m + 1) * 128], id32_sb[0:32, 0:32])
            nmt = wpool.tile([128, MC], F32, tag="nmt")
            et = wpool.tile([128, MC * CLS], F32, tag="et")
            st = wpool.tile([128, MC], F32, tag="st")
            lst = wpool.tile([128, MC], F32, tag="lst")
            nc.vector.reduce_max(
                nmt[:, :],
                ztp[:].rearrange("p (m f) -> p m f", m=MC)[:, :, 0:CLS],
                axis=mybir.AxisListType.X, negate=True)
            zs = wpool.tile([128, MC * CLS], F32, tag="zs")
            for m in range(MC):
                nc.vector.tensor_scalar_add(
                    zs[:, m * CLS:(m + 1) * CLS],
                    ztp[:, m * 32: m * 32 + CLS], nmt[:, m:m + 1])
            nc.scalar.activation(et[:, :], zs[:, :], AF.Exp)
            nc.vector.reduce_sum(
                st[:, :], et[:].rearrange("p (m f) -> p m f", m=MC),
                axis=mybir.AxisListType.X)
            nc.scalar.activation(lst[:, :], st[:, :], AF.Ln)
            for m in range(MC):
                nc.vector.tensor_scalar(
                    outsb[:, m * CLS:(m + 1) * CLS],
                    ztp[:, m * 32: m * 32 + CLS],
                    nmt[:, m:m + 1], lst[:, m:m + 1],
                    op0=AG.add, op1=AG.subtract)
            nc.scalar.dma_start(
                out.ap().rearrange("(m p) f -> p m f", p=128),
                outsb[:].rearrange("p (m f) -> p m f", m=MC))

    nc.compile()
    return nc


# ---------------------------------------------------------------------------
# host-side preprocessing
# ---------------------------------------------------------------------------

def _preprocess(node_feats, edge_index, W1, b1, W12, b12, W13, b13, W14, b14,
                W2, b2):
    src = np.asarray(edge_index[0], dtype=np.int64)
    dst = np.asarray(edge_index[1], dtype=np.int64)

    # dense-path matrix: B[i,j] = #edges(i->j) offdiag, diag forced to 1
    Bden = np.zeros(NP * NP, dtype=np.uint8)
    np.add.at(Bden, src * NP + dst, 1)
    Bden = Bden.reshape(NP, NP)
    idx = np.arange(N)
    Bden[idx, idx] = 1
    deg_den = Bden[:N].sum(axis=1, dtype=np.int64).astype(np.float64)
    dis = np.zeros(NP, dtype=np.float64)
    dis[:N] = np.maximum(deg_den, 1.0) ** -0.5
    dis[N:] = 1.0

    # sparse-path matrix: Bsp[t,s] = #edges(s->t) + I
    Bsp = np.zeros(NP * NP, dtype=np.uint8)
    np.add.at(Bsp, dst * NP + src, 1)
    Bsp = Bsp.reshape(NP, NP)
    Bsp[idx, idx] += 1
    deg_sp = Bsp[:N].sum(axis=1, dtype=np.int64).astype(np.float64)
    dinv = np.zeros(NP, dtype=np.float64)
    dinv[:N] = np.where(deg_sp > 0, deg_sp.astype(np.float64) ** -0.5, 0.0)

    x0 = np.zeros((NP, F_IN), dtype=np.float32)
    x0[:N] = np.asarray(node_feats, dtype=np.float32)

    def pp(vec, c):
        loc = vec[c * NLOC:(c + 1) * NLOC].astype(np.float32)
        return np.ascontiguousarray(loc.reshape(MC, 128).T)

    def repl(vec, c):
        loc = vec[c * NLOC:(c + 1) * NLOC].astype(np.float32)
        return np.ascontiguousarray(np.broadcast_to(loc[None, :], (128, NLOC)))

    def pack_b(B, rows):
        # [s, t_local] chunked over s, in CHUNK_ORDER, group-major so each
        # DMA group is one fully contiguous block of DRAM.
        bt = B[rows].T.reshape(KC, 128, NLOC)[CHUNK_ORDER]
        bt = bt.reshape(NGRP, GC, 128, NLOC).transpose(0, 2, 1, 3)
        return np.ascontiguousarray(bt.reshape(NGRP, 128, GW)).astype(NP_F8)

    w12b = np.concatenate([np.asarray(W12, np.float32),
                           np.asarray(b12, np.float32)[None, :]], axis=0)
    w13b = np.concatenate([np.asarray(W13, np.float32),
                           np.asarray(b13, np.float32)[None, :]], axis=0)
    biases_pp = np.zeros((128, 3), dtype=np.float32)
    biases_pp[:32, 0] = np.asarray(b1, np.float32)
    biases_pp[:, 1] = np.asarray(b14, np.float32)
    biases_pp[:CLS, 2] = np.asarray(b2, np.float32)

    in_maps = []
    for c in range(NCORES):
        rows = slice(c * NLOC, (c + 1) * NLOC)
        featT_c = np.ascontiguousarray(x0[rows].T).reshape(4, 128, NLOC)
        in_maps.append({
            "bden": pack_b(Bden, rows),
            "bsp": pack_b(Bsp, rows),
            "featT": featT_c.astype(NP_F16),
            "w1": np.asarray(W1, np.float32).reshape(4, 128, 32).astype(NP_F16),
            "w12b": w12b.astype(NP_F16),
            "w13b": w13b.astype(NP_F16),
            "w14": np.asarray(W14, np.float32).astype(NP_F16),
            "w2": np.asarray(W2, np.float32).astype(NP_F16),
            "biases_pp": biases_pp,
            "dis_repl": repl(dis, c),
            "dinv_repl": repl(dinv, c),
            "dis_pp": pp(dis, c),
            "dinv_pp": pp(dinv, c),
            "ident16": np.eye(128, dtype=NP_F16),
            "ident32": np.eye(128, dtype=np.float32),
        })
    return in_maps


def kernel(node_feats, edge_index, W1, b1, W12, b12, W13, b13, W14, b14, W2,
           b2):
    in_maps = _preprocess(node_feats, edge_index, W1, b1, W12, b12, W13, b13,
                          W14, b14, W2, b2)
    if "nc" not in _cached:
        _cached["nc"] = _build_program()
    nc = _cached["nc"]
    trace = bool(int(os.environ.get("KERNEL_TRACE", "0")))
    res = run_bass_kernel_spmd(nc, in_maps, core_ids=list(range(NCORES)),
                               trace=trace)
    _cached["last_result"] = res
    outs = [res.results[c]["out"] for c in range(NCORES)]
    return np.concatenate(outs, axis=0)[:N].astype(np.float32)


# revision 19
# speedup vs baseline: 1.0798x; 1.0798x over previous
"""GCN (2 dense + 3 sparse layers + log_softmax) on 8 Trainium2 NeuronCores.

Strategy: each graph aggregation A_norm @ H runs densely on the PE as
out_T[f, t] = sum_s H'[s, f] * B^T[s, t] with B the count-valued adjacency in
fp8 (exact small integers) and H' the diag-scaled features in fp16
(stationary operand).  Nodes (dst) are row-sharded 8 ways; each layer's
feature block is exchanged via AllGather (half-exchanges for the 32-wide
layers where Mesh latency is low; one exchange for the wider layers).

The two adjacency matrices are RESIDENT in SBUF: bden streams from HBM once
(used by l1 + l2), then bsp streams once into the same region as l2
consumes it (used by l3/l4/l5).  All h_full fill DMAs ride the same sync
HWDGE queue as the B streams, interleaved so small fill packets are never
starved by stream packets on the shared SDMA engines.
"""

import os
import numpy as np
import ml_dtypes

import concourse.bacc as bacc
import concourse.mybir as mybir
import concourse.tile as tile
from concourse.bass_utils import run_bass_kernel_spmd

# ---- problem constants ----
N = 12000
NP = 12288         # padded nodes (96 * 128)
NCORES = 8
NLOC = NP // NCORES            # 1536 rows per core
KC = NP // 128                 # 96 k-chunks
MC = NLOC // 128               # 12 local row chunks
MH = MC // 2                   # half split (6 chunks)
NT = NLOC // 512               # 3 psum col tiles
NGRP = 8                       # B-stream DMA groups
GC = KC // NGRP                # 12 chunks per group
GW = GC * NLOC                 # group width in cden columns
F_IN = 512
CLS = 6

F8 = mybir.dt.float8e4
F16 = mybir.dt.float16
F32 = mybir.dt.float32
NP_F8 = ml_dtypes.float8_e4m3
NP_F16 = np.float16

D1, D2, D3, D4, D5 = 32, 32, 64, 128, 32   # aggregation widths per layer

# B consumption order: A-half (m<MH of every rank) first, then B-half, so
# each layer's second half fill hides under first-half aggregation.
# B matrices are stored in this order host-side -> sequential DMA stream.
CHUNK_ORDER = ([c * MC + m for c in range(NCORES) for m in range(MH)]
               + [c * MC + m for c in range(NCORES) for m in range(MH, MC)])

_cached = {}


def _build_program():
    nc = bacc.Bacc("TRN2", target_bir_lowering=False, debug=False,
                   num_devices=NCORES)

    bden = nc.dram_tensor("bden", [NGRP, 128, GW], F8, kind="ExternalInput")
    bsp = nc.dram_tensor("bsp", [NGRP, 128, GW], F8, kind="ExternalInput")
    featT = nc.dram_tensor("featT", [4, 128, NLOC], F16, kind="ExternalInput")
    w1 = nc.dram_tensor("w1", [4, 128, 32], F16, kind="ExternalInput")
    w12b = nc.dram_tensor("w12b", [33, 64], F16, kind="ExternalInput")
    w13b = nc.dram_tensor("w13b", [65, 128], F16, kind="ExternalInput")
    w14 = nc.dram_tensor("w14", [128, 128], F16, kind="ExternalInput")
    w2 = nc.dram_tensor("w2", [128, CLS], F16, kind="ExternalInput")
    biases_pp = nc.dram_tensor("biases_pp", [128, 3], F32, kind="ExternalInput")
    dis_repl = nc.dram_tensor("dis_repl", [128, NLOC], F32, kind="ExternalInput")
    dinv_repl = nc.dram_tensor("dinv_repl", [128, NLOC], F32, kind="ExternalInput")
    dis_pp = nc.dram_tensor("dis_pp", [128, MC], F32, kind="ExternalInput")
    dinv_pp = nc.dram_tensor("dinv_pp", [128, MC], F32, kind="ExternalInput")
    ident16 = nc.dram_tensor("ident16", [128, 128], F16, kind="ExternalInput")
    ident32 = nc.dram_tensor("ident32", [128, 128], F32, kind="ExternalInput")
    out = nc.dram_tensor("out", [NLOC, CLS], F32, kind="ExternalOutput")

    AG = mybir.AluOpType
    AF = mybir.ActivationFunctionType
    RG = [list(range(NCORES))]

    with tile.TileContext(nc) as tc:
        with (
            tc.tile_pool(name="const", bufs=1) as cpool,
            tc.tile_pool(name="dscale", bufs=1) as dpool_s,
            tc.tile_pool(name="resident", bufs=1) as hpool,
            tc.tile_pool(name="fq", bufs=2) as fpool,
            tc.tile_pool(name="work", bufs=1) as wpool,
            tc.tile_pool(name="post", bufs=3) as ppool,
            tc.tile_pool(name="small", bufs=2) as spool,
            tc.tile_pool(name="agg", bufs=4, space="PSUM") as aggp,
            tc.tile_pool(name="wmm", bufs=2, space="PSUM") as wmmp,
            tc.tile_pool(name="tp", bufs=1, space="PSUM") as tpp,
            tc.tile_pool(name="dram", bufs=1, space="DRAM") as dpool,
        ):
            # ---------- early constants (l1 path only) ----------
            w1_sb = cpool.tile([128, 4 * 32], F16, tag="w1")
            nc.scalar.dma_start(w1_sb[:].rearrange("p (c j) -> p c j", c=4),
                                w1.ap().rearrange("c p j -> p c j"))
            bias_sb = cpool.tile([128, 3], F32, tag="bias")
            nc.scalar.dma_start(bias_sb[:], biases_pp[:, :])
            dispp_sb = cpool.tile([128, MC], F32, tag="dispp")
            nc.scalar.dma_start(dispp_sb[:], dis_pp[:, :])
            id16_sb = cpool.tile([128, 128], F16, tag="id16")
            nc.scalar.dma_start(id16_sb[:], ident16[:, :])

            # resident adjacency (bden first, overwritten by bsp during l2)
            cden = hpool.tile([128, KC * NLOC], F8, tag="cden")
            # h_full holds up to 64 feature cols per chunk; layer 4 (d=128)
            # runs as two 64-wide f-half passes re-filled from DRAM.
            h_full = hpool.tile([128, KC * 64], F16, tag="hfull")

            def m_exchange(hloc, d, lname, tagid, m0, nm):
                """AllGather nm chunks (m0..m0+nm) of the local block;
                returns the gathered DRAM buffer."""
                w = nm * d
                bin_t = dpool.tile([128, w], F16, tag=f"agi{lname}{tagid}")
                bout_t = dpool.tile([NCORES, 128, w], F16,
                                    tag=f"ago{lname}{tagid}",
                                    addr_space="Shared")
                nc.scalar.dma_start(bin_t[:], hloc[:, m0 * d:m0 * d + w])
                nc.gpsimd.collective_compute(
                    "AllGather", AG.bypass, replica_groups=RG,
                    ins=[bin_t.opt()], outs=[bout_t.opt()],
                )
                return bout_t

            def fill_hfull(bout_t, d_src, m0, nm, f0, fw, src_m0=0):
                """Scatter gathered blocks into h_full chunk positions
                c*MC+m0.. (fw cols per chunk), source cols f0:f0+fw of a
                d_src-wide gather starting at chunk src_m0 inside bout.
                Rides the sync HWDGE queue (full rate vs streams)."""
                view = h_full[:, 0:KC * fw].rearrange("p (c x) -> p c x",
                                                      c=NCORES)
                dst = view[:, :, m0 * fw:(m0 + nm) * fw]
                mo = src_m0 * d_src
                if f0 == 0 and fw == d_src:
                    return nc.sync.dma_start(
                        dst, bout_t[:, :, mo:mo + nm * fw].rearrange(
                            "c p w -> p c w"))
                else:
                    # f-sliced case needs 4D APs, unsupported by DMA ->
                    # one 3D transfer per source core.
                    last = None
                    for c in range(NCORES):
                        k0 = c * MC + m0
                        last = nc.sync.dma_start(
                            h_full[:, k0 * fw:(k0 + nm) * fw].rearrange(
                                "p (m f) -> p m f", f=fw),
                            bout_t[c, :, mo:mo + nm * d_src].rearrange(
                                "p (m f) -> p m f",
                                f=d_src)[:, :, f0:f0 + fw])
                    return last

            def bpass(d, lname, order=None):
                """Aggregation pass over the resident adjacency, consuming
                stream positions in CHUNK_ORDER (A-half chunks first)."""
                P4 = 128 // d    # col-group packing factor
                aggs = [aggp.tile([128, 512], F32, tag="agg",
                                  name=f"agg_{lname}_{i}") for i in range(NT)]
                positions = order if order is not None else range(KC)
                for seq, i in enumerate(positions):
                    k = CHUNK_ORDER[i]      # global chunk at stream pos i
                    q = seq % P4             # PE column group
                    lhs = h_full[:, k * d:(k + 1) * d]
                    for t in range(NT):
                        nc.tensor.matmul(
                            aggs[t][q * d:(q + 1) * d, :], lhs,
                            cden[:, i * NLOC + t * 512:
                                 i * NLOC + (t + 1) * 512],
                            start=(seq < P4), stop=(seq >= KC - P4),
                            tile_position=(0, q * d),
                        )
                return aggs

            def gsum(aggs, t, d, lname, pbase=0):
                """Sum the P4 col-group partials of psum tile t -> [d, 512]
                placed at SBUF partitions pbase:pbase+d (SB operands of a
                DVE op must share a start partition; PSUM ones are free)."""
                P4 = 128 // d
                a = aggs[t]
                if P4 == 1:
                    return a
                tmp = spool.tile([128, 512], F32, tag="gsum",
                                 name=f"gs_{lname}_{t}")
                view = tmp[pbase:pbase + d, :]
                nc.vector.tensor_copy(view, a[pbase:pbase + d, :])
                for q in range(P4):
                    if q * d == pbase:
                        continue
                    nc.vector.tensor_tensor(view, view,
                                            a[q * d:(q + 1) * d, :], op=AG.add)
                return tmp[pbase:pbase + d, :]

            # bden stream starts immediately; l1 consumes groups as they land
            for g in range(NGRP):
                nc.sync.dma_start(cden[:, g * GW:(g + 1) * GW], bden[g, :, :])

            # ============ L1 local transform: H'1 = dis * (X0 @ W1) ==========
            # kc-outer so the feature matrix streams in quarters (6 KB SBUF).
            # Each (kc, m) matmul is a self-contained group into its own psum
            # column range; the kc-reduction is summed on the vector engine.
            h1loc = wpool.tile([128, MC * D1], F16, tag="h1loc")
            t1k = []
            for kc in range(4):
                fq = fpool.tile([128, NLOC], F16, tag="fq", name=f"fq{kc}")
                nc.scalar.dma_start(fq[:], featT[kc, :, :])
                tk = aggp.tile([128, MC * 32], F32, tag="agg", name=f"t1k{kc}")
                t1k.append(tk)
                for m in range(MC):
                    nc.tensor.matmul(
                        tk[:, m * 32:(m + 1) * 32],
                        fq[:, m * 128:(m + 1) * 128],
                        w1_sb[:, kc * 32:(kc + 1) * 32],
                        start=True, stop=True,
                    )
            bouts = {}
            for half in range(2):
                cols = slice(half * MH * 32, (half + 1) * MH * 32)
                s01 = spool.tile([128, MH * 32], F32, tag="t1s",
                                 name=f"s01_{half}")
                nc.vector.tensor_copy(s01[:, :], t1k[0][:, cols])
                for kc in range(1, 4):
                    nc.vector.tensor_tensor(s01[:, :], s01[:, :],
                                            t1k[kc][:, cols], op=AG.add)
                for m in range(half * MH, (half + 1) * MH):
                    col = (m % MH) * 32
                    nc.vector.tensor_scalar_mul(
                        h1loc[:, m * D1:(m + 1) * D1],
                        s01[:, col:col + 32],
                        dispp_sb[:, m:m + 1])
                bouts[("l1", half)] = m_exchange(h1loc, D1, "l1", half,
                                                 half * MH, MH)
            for half in range(2):
                fill_hfull(bouts[("l1", half)], D1, half * MH, MH, 0, D1)

            # dis/dinv replicated scale rows (loaded in the quiet window
            # before the bsp stream starts)
            disr_sb = dpool_s.tile([128, NLOC], F32, tag="dsc", name="disr")
            nc.scalar.dma_start(disr_sb[:], dis_repl[:, :])
            dinvr_sb = dpool_s.tile([128, NLOC], F32, tag="dinv", name="dinvr")
            nc.scalar.dma_start(dinvr_sb[:], dinv_repl[:, :])

            # ============ L1 agg + post: x1 = relu(dis*G1 + b1) ==============
            aggs = bpass(D1, "l1")
            x1p = ppool.tile([32, NLOC], F16, tag="post", name="x1p")
            h2loc = wpool.tile([128, MC * D2], F16, tag="h2loc")
            tp1 = tpp.tile([128, MC * 32], F16, tag="tp16")
            for half in range(2):
                for t in ((0, 1) if half == 0 else (2,)):
                    sl = slice(t * 512, (t + 1) * 512)
                    g1s = gsum(aggs, t, D1, "l1")
                    nc.vector.tensor_tensor(
                        g1s[:, :], g1s[:, :], disr_sb[0:32, sl], op=AG.mult)
                    x1t = spool.tile([32, 512], F16, tag="x1t", name=f"x1t_{t}")
                    nc.scalar.activation(x1t[:, :], g1s[:, :], AF.Relu,
                                         bias=bias_sb[0:32, 0:1])
                    nc.vector.tensor_tensor(
                        x1p[:, sl], x1t[:, :], disr_sb[0:32, sl], op=AG.mult)
                for m in range(half * MH, (half + 1) * MH):
                    nc.tensor.transpose(
                        tp1[:, m * 32:(m + 1) * 32],
                        x1p[:, m * 128:(m + 1) * 128], id16_sb[0:32, 0:32])
                o = half * MH * D2
                nc.vector.tensor_copy(h2loc[:, o:o + MH * D2],
                                      tp1[:, o:o + MH * D2])
                bouts[("l2", half)] = m_exchange(h2loc, D2, "l2", half,
                                                 half * MH, MH)
            l2_fill_last = None
            for half in range(2):
                l2_fill_last = fill_hfull(bouts[("l2", half)], D2,
                                          half * MH, MH, 0, D2)

            # ============ L2: agg + x2 = relu(dis*G2 @ W12 + b12) ============
            w12_sb = cpool.tile([33, 64], F16, tag="w12")
            nc.scalar.dma_start(w12_sb[:], w12b[:, :])
            dinvpp_sb = cpool.tile([128, MC], F32, tag="dinvpp")
            nc.scalar.dma_start(dinvpp_sb[:], dinv_pp[:, :])
            aggs = bpass(D2, "l2")
            # bsp overwrites each cden group as soon as l2's matmuls have
            # consumed it (WAR deps per slice).  Groups are emitted in
            # slices with the l3 fills interleaved between them, so the
            # fills get full SDMA bandwidth at the moment they unblock.
            for g in range(4):
                bg = nc.sync.dma_start(cden[:, g * GW:(g + 1) * GW],
                                       bsp[g, :, :])
                if g == 0:
                    # keep the tiny l2 fills ahead of the stream on the
                    # shared SDMA engines (else they starve ~10us)
                    tile.add_dep_helper(l2_fill_last.ins, bg.ins, sync=True,
                                        reason="l2 fills before bsp stream")
            g2p = ppool.tile([33, NLOC], F16, tag="post", name="g2p")
            nc.vector.memset(g2p[32:33, :], 1.0)
            b3qs = []
            h3loc = wpool.tile([128, MC * D3], F16, tag="h3loc")
            for half in range(2):
                for t in ((0, 1) if half == 0 else (2,)):
                    sl = slice(t * 512, (t + 1) * 512)
                    nc.vector.tensor_tensor(
                        g2p[0:32, sl], gsum(aggs, t, D2, "l2"), disr_sb[0:32, sl],
                        op=AG.mult)
                for m in range(half * MH, (half + 1) * MH):
                    xp = wmmp.tile([128, 64], F32, tag="wmm", name=f"x2_{m}")
                    nc.tensor.matmul(xp[:, :], g2p[:, m * 128:(m + 1) * 128],
                                     w12_sb[:, :], start=True, stop=True)
                    nc.vector.tensor_scalar(
                        h3loc[:, m * D3:(m + 1) * D3], xp[:, :],
                        0.0, dinvpp_sb[:, m:m + 1], op0=AG.max, op1=AG.mult)
                # l3 exchange in 49 KB quarters: each picks the low-latency
                # Mesh algorithm; its fill unblocks a quarter of the pass
                for q in (2 * half, 2 * half + 1):
                    b3qs.append(m_exchange(h3loc, D3, "l3", q, q * 3, 3))
            for q in range(4):
                fill_hfull(b3qs[q], D3, q * 3, 3, 0, D3)
            for g in range(4, NGRP):
                nc.sync.dma_start(cden[:, g * GW:(g + 1) * GW], bsp[g, :, :])

            # ============ L3: agg + x3 = relu(dinv*G3 @ W13 + b13) ===========
            w13_sb = cpool.tile([65, 128], F16, tag="w13")
            nc.scalar.dma_start(w13_sb[:], w13b[:, :])
            l3_order = sorted(range(KC), key=lambda i: (
                (CHUNK_ORDER[i] % MC) // 3, i))
            aggs = bpass(D3, "l3", order=l3_order)
            g3p = ppool.tile([65, NLOC], F16, tag="post", name="g3p")
            nc.vector.memset(g3p[64:65, :], 1.0)
            h4loc = wpool.tile([128, MC * D4], F16, tag="h4loc")
            for half in range(2):
                for t in ((0, 1) if half == 0 else (2,)):
                    sl = slice(t * 512, (t + 1) * 512)
                    nc.vector.tensor_tensor(
                        g3p[0:64, sl], gsum(aggs, t, D3, "l3"), dinvr_sb[0:64, sl],
                        op=AG.mult)
                for m in range(half * MH, (half + 1) * MH):
                    xp = wmmp.tile([128, 128], F32, tag="wmm", name=f"x3_{m}")
                    nc.tensor.matmul(xp[:, :], g3p[:, m * 128:(m + 1) * 128],
                                     w13_sb[:, :], start=True, stop=True)
                    nc.vector.tensor_scalar(
                        h4loc[:, m * D4:(m + 1) * D4], xp[:, :],
                        0.0, dinvpp_sb[:, m:m + 1], op0=AG.max, op1=AG.mult)
            # one whole-block exchange for l4 (f-halves filled separately)
            bout4 = m_exchange(h4loc, D4, "l4", 0, 0, MC)
            for half in range(2):
                fill_hfull(bout4, D4, half * MH, MH, 0, 64,
                           src_m0=half * MH)

            # ===== L4: agg + x4T = relu(dinv*G4 @ W14 + b14)  (transposed) ===
            # ===== L5a: H'5T = dinv * (x4 @ W2), transpose, exchange =========
            # d=128 runs as two 64-wide f-half passes so h_full stays 12 KB.
            w14_sb = cpool.tile([128, 128], F16, tag="w14")
            nc.scalar.dma_start(w14_sb[:], w14[:, :])
            w2_sb = cpool.tile([128, CLS], F16, tag="w2")
            nc.scalar.dma_start(w2_sb[:], w2[:, :])
            aggs = bpass(64, "l4a")
            g4p = ppool.tile([128, NLOC], F16, tag="post", name="g4p")
            for t in range(NT):
                sl = slice(t * 512, (t + 1) * 512)
                nc.vector.tensor_tensor(
                    g4p[0:64, sl], gsum(aggs, t, 64, "l4a"),
                    dinvr_sb[0:64, sl], op=AG.mult)
            for half in range(2):
                fill_hfull(bout4, D4, half * MH, MH, 64, 64,
                           src_m0=half * MH)
            aggs = bpass(64, "l4b")
            x4T = ppool.tile([128, NLOC], F16, tag="post", name="x4T")
            h5T = ppool.tile([32, NLOC], F16, tag="post", name="h5T")
            nc.vector.memset(h5T[0:32, :], 0.0)
            h5loc = wpool.tile([128, MC * D5], F16, tag="h5loc")
            tp5 = tpp.tile([128, MC * 32], F16, tag="tp16")
            for half in range(2):
                for t in ((0, 1) if half == 0 else (2,)):
                    sl = slice(t * 512, (t + 1) * 512)
                    nc.vector.tensor_tensor(
                        g4p[64:128, sl], gsum(aggs, t, 64, "l4b", pbase=64),
                        dinvr_sb[64:128, sl], op=AG.mult)
                    x4p = wmmp.tile([128, 512], F32, tag="wmm", name=f"x4_{t}")
                    nc.tensor.matmul(x4p[:, :], w14_sb[:, :], g4p[:, sl],
                                     start=True, stop=True)
                    nc.scalar.activation(x4T[:, sl], x4p[:, :], AF.Relu,
                                         bias=bias_sb[:, 1:2])
                    t5 = wmmp.tile([CLS, 512], F32, tag="wmm", name=f"t5_{t}")
                    nc.tensor.matmul(t5[:, :], w2_sb[:, :], x4T[:, sl],
                                     start=True, stop=True)
                    nc.vector.tensor_tensor(
                        h5T[0:CLS, sl], t5[:, :], dinvr_sb[0:CLS, sl],
                        op=AG.mult)
                for m in range(half * MH, (half + 1) * MH):
                    nc.tensor.transpose(
                        tp5[:, m * 32:(m + 1) * 32],
                        h5T[:, m * 128:(m + 1) * 128], id16_sb[0:32, 0:32])
                o = half * MH * D5
                nc.vector.tensor_copy(h5loc[:, o:o + MH * D5],
                                      tp5[:, o:o + MH * D5])
                bouts[("l5", half)] = m_exchange(h5loc, D5, "l5", half,
                                                 half * MH, MH)
            for half in range(2):
                fill_hfull(bouts[("l5", half)], D5, half * MH, MH, 0, D5)

            # ============ L5b: agg + z = dinv*G5 + b2, log_softmax ===========
            id32_sb = cpool.tile([128, 128], F32, tag="id32")
            nc.scalar.dma_start(id32_sb[:], ident32[:, :])
            aggs = bpass(D5, "l5")
            zt = wpool.tile([32, NLOC], F32, tag="zt")
            nc.vector.memset(zt[0:32, :], 0.0)
            for t in range(NT):
                sl = slice(t * 512, (t + 1) * 512)
                nc.vector.tensor_tensor(
                    zt[0:CLS, sl], gsum(aggs, t, D5, "l5")[0:CLS, :],
                    dinvr_sb[0:CLS, sl], op=AG.mult)
                nc.vector.tensor_scalar_add(
                    zt[0:CLS, sl], zt[0:CLS, sl], bias_sb[0:CLS, 2:3])
            ztp = tpp.tile([128, MC * 32], F32, tag="tp32")
            outsb = wpool.tile([128, MC * CLS], F32, tag="outsb")
            for m in range(MC):
                nc.tensor.transpose(
                    ztp[:, m * 32:(m + 1) * 32],
                    zt[:, m * 128:(m + 1) * 128], id32_sb[0:32, 0:32])
            nmt = wpool.tile([128, MC], F32, tag="nmt")
            et = wpool.tile([128, MC * CLS], F32, tag="et")
            st = wpool.tile([128, MC], F32, tag="st")
            lst = wpool.tile([128, MC], F32, tag="lst")
            nc.vector.reduce_max(
                nmt[:, :],
                ztp[:].rearrange("p (m f) -> p m f", m=MC)[:, :, 0:CLS],
                axis=mybir.AxisListType.X, negate=True)
            zs = wpool.tile([128, MC * CLS], F32, tag="zs")
            for m in range(MC):
                nc.vector.tensor_scalar_add(
                    zs[:, m * CLS:(m + 1) * CLS],
                    ztp[:, m * 32: m * 32 + CLS], nmt[:, m:m + 1])
            nc.scalar.activation(et[:, :], zs[:, :], AF.Exp)
            nc.vector.reduce_sum(
                st[:, :], et[:].rearrange("p (m f) -> p m f", m=MC),
                axis=mybir.AxisListType.X)
            nc.scalar.activation(lst[:, :], st[:, :], AF.Ln)
            for m in range(MC):
                nc.vector.tensor_scalar(
                    outsb[:, m * CLS:(m + 1) * CLS],
                    ztp[:, m * 32: m * 32 + CLS],
                    nmt[:, m:m + 1], lst[:, m:m + 1],
                    op0=AG.add, op1=AG.subtract)
            nc.scalar.dma_start(
                out.ap().rearrange("(m p) f -> p m f", p=128),
                outsb[:].rearrange("p (m f) -> p m f", m=MC))

    nc.compile()
    return nc


# ---------------------------------------------------------------------------
# host-side preprocessing
# ---------------------------------------------------------------------------

def _preprocess(node_feats, edge_index, W1, b1, W12, b12, W13, b13, W14, b14,
                W2, b2):
    src = np.asarray(edge_index[0], dtype=np.int64)
    dst = np.asarray(edge_index[1], dtype=np.int64)

    # dense-path matrix: B[i,j] = #edges(i->j) offdiag, diag forced to 1
    Bden = np.zeros(NP * NP, dtype=np.uint8)
    np.add.at(Bden, src * NP + dst, 1)
    Bden = Bden.reshape(NP, NP)
    idx = np.arange(N)
    Bden[idx, idx] = 1
    deg_den = Bden[:N].sum(axis=1, dtype=np.int64).astype(np.float64)
    dis = np.zeros(NP, dtype=np.float64)
    dis[:N] = np.maximum(deg_den, 1.0) ** -0.5
    dis[N:] = 1.0

    # sparse-path matrix: Bsp[t,s] = #edges(s->t) + I
    Bsp = np.zeros(NP * NP, dtype=np.uint8)
    np.add.at(Bsp, dst * NP + src, 1)
    Bsp = Bsp.reshape(NP, NP)
    Bsp[idx, idx] += 1
    deg_sp = Bsp[:N].sum(axis=1, dtype=np.int64).astype(np.float64)
    dinv = np.zeros(NP, dtype=np.float64)
    dinv[:N] = np.where(deg_sp > 0, deg_sp.astype(np.float64) ** -0.5, 0.0)

    x0 = np.zeros((NP, F_IN), dtype=np.float32)
    x0[:N] = np.asarray(node_feats, dtype=np.float32)

    def pp(vec, c):
        loc = vec[c * NLOC:(c + 1) * NLOC].astype(np.float32)
        return np.ascontiguousarray(loc.reshape(MC, 128).T)

    def repl(vec, c):
        loc = vec[c * NLOC:(c + 1) * NLOC].astype(np.float32)
        return np.ascontiguousarray(np.broadcast_to(loc[None, :], (128, NLOC)))

    def pack_b(B, rows):
        # [s, t_local] chunked over s, in CHUNK_ORDER, group-major so each
        # DMA group is one fully contiguous block of DRAM.
        bt = B[rows].T.reshape(KC, 128, NLOC)[CHUNK_ORDER]
        bt = bt.reshape(NGRP, GC, 128, NLOC).transpose(0, 2, 1, 3)
        return np.ascontiguousarray(bt.reshape(NGRP, 128, GW)).astype(NP_F8)

    w12b = np.concatenate([np.asarray(W12, np.float32),
                           np.asarray(b12, np.float32)[None, :]], axis=0)
    w13b = np.concatenate([np.asarray(W13, np.float32),
                           np.asarray(b13, np.float32)[None, :]], axis=0)
    biases_pp = np.zeros((128, 3), dtype=np.float32)
    biases_pp[:32, 0] = np.asarray(b1, np.float32)
    biases_pp[:, 1] = np.asarray(b14, np.float32)
    biases_pp[:CLS, 2] = np.asarray(b2, np.float32)

    in_maps = []
    for c in range(NCORES):
        rows = slice(c * NLOC, (c + 1) * NLOC)
        featT_c = np.ascontiguousarray(x0[rows].T).reshape(4, 128, NLOC)
        in_maps.append({
            "bden": pack_b(Bden, rows),
            "bsp": pack_b(Bsp, rows),
            "featT": featT_c.astype(NP_F16),
            "w1": np.asarray(W1, np.float32).reshape(4, 128, 32).astype(NP_F16),
            "w12b": w12b.astype(NP_F16),
            "w13b": w13b.astype(NP_F16),
            "w14": np.asarray(W14, np.float32).astype(NP_F16),
            "w2": np.asarray(W2, np.float32).astype(NP_F16),
            "biases_pp": biases_pp,
            "dis_repl": repl(dis, c),
            "dinv_repl": repl(dinv, c),
            "dis_pp": pp(dis, c),
            "dinv_pp": pp(dinv, c),
            "ident16": np.eye(128, dtype=NP_F16),
            "ident32": np.eye(128, dtype=np.float32),
        })
    return in_maps


def kernel(node_feats, edge_index, W1, b1, W12, b12, W13, b13, W14, b14, W2,
           b2):
    in_maps = _preprocess(node_feats, edge_index, W1, b1, W12, b12, W13, b13,
                          W14, b14, W2, b2)
    if "nc" not in _cached:
        _cached["nc"] = _build_program()
    nc = _cached["nc"]
    trace = bool(int(os.environ.get("KERNEL_TRACE", "0")))
    res = run_bass_kernel_spmd(nc, in_maps, core_ids=list(range(NCORES)),
                               trace=trace)
    _cached["last_result"] = res
    outs = [res.results[c]["out"] for c in range(NCORES)]
    return np.concatenate(outs, axis=0)[:N].astype(np.float32)
